# revision 1
# baseline (speedup 1.0000x reference)
"""Mask R-CNN paste_masks_in_image kernel for Trainium2 (8 NeuronCores).

out[n] = Y_n @ mask_n @ X_n   (separable bilinear paste, all f32)

 - X_n [28, img_w]: column-interp matrix, <=2 nonzeros per column
 - Y_n [img_h, 28]: row-interp matrix, <=2 nonzeros per row
 - Host builds X / Y^T from boxes (tiny), device does the matmuls and
   writes the full [N, img_h, img_w] output. Data-parallel over N: 16
   instances per core on 8 cores.
"""
import sys

if "/opt/trn_rl_repo" not in sys.path:
    sys.path.insert(0, "/opt/trn_rl_repo")

import numpy as np

N_CORES = 8
HM = WM = 28


# ---------------------------------------------------------------------------
# walrus in this image allows only ONE sync-wait per instruction; split any
# instruction carrying N>1 waits into N-1 preceding NoOps on the same engine.
_ws_ctr = [0]


def _split_multi_waits(nc):
    import concourse.mybir as mybir

    for fn in nc.m.functions:
        for blk in fn.blocks:
            insts = list(blk.instructions)
            out = []
            changed = False
            for inst in insts:
                si = getattr(inst, "sync_info", None)
                waits = list(si.on_wait) if (si is not None and si.on_wait) else []
                if len(waits) > 1:
                    changed = True
                    for w in waits[:-1]:
                        _ws_ctr[0] += 1
                        out.append(
                            mybir.InstNoOp(
                                name=f"waitsplit-{_ws_ctr[0]}",
                                engine=inst.engine,
                                sync_info=mybir.SyncInfo(on_wait=[w], on_update=[]),
                            )
                        )
                    si.on_wait = [waits[-1]]
                out.append(inst)
            if changed:
                try:
                    blk.instructions = out
                except Exception:
                    del blk.instructions[:]
                    blk.instructions.extend(out)


# ---------------------------------------------------------------------------
# Host-side: interp weight matrices (exact f32 replication of the reference).
def _interp_mats(p0, p1, out_size, mask_size):
    """p0/p1: [N] f32 box edges (already scaled+clipped). Returns W [N, mask_size,
    out_size] f32 with W[n, k, j] = w0*(i0==k) + w1*(i0+1==k)."""
    n = p0.shape[0]
    xs = (np.arange(out_size, dtype=np.float32) + np.float32(0.5))[None, :]
    g = (xs - p0[:, None]) / (p1 - p0)[:, None] * np.float32(2) - np.float32(1)
    p = (g + np.float32(1)) * np.float32(mask_size * 0.5) - np.float32(0.5)
    f = np.floor(p)
    i0 = f.astype(np.int64)
    w1 = (p - f).astype(np.float32)
    w0 = np.float32(1.0) - w1
    ks = np.arange(mask_size, dtype=np.int64)[None, :, None]
    W = (i0[:, None, :] == ks) * w0[:, None, :] + ((i0 + 1)[:, None, :] == ks) * w1[
        :, None, :
    ]
    return np.ascontiguousarray(W.astype(np.float32))


def _scaled_boxes(boxes, img_h, img_w, in_h, in_w):
    sx = np.float32(img_w / in_w)
    sy = np.float32(img_h / in_h)
    b = boxes.astype(np.float32) * np.array([sx, sy, sx, sy], np.float32)
    x0 = np.clip(b[:, 0], np.float32(0.0), np.float32(img_w))
    y0 = np.clip(b[:, 1], np.float32(0.0), np.float32(img_h))
    x1 = np.clip(b[:, 2], np.float32(0.0), np.float32(img_w))
    y1 = np.clip(b[:, 3], np.float32(0.0), np.float32(img_h))
    return x0, y0, x1, y1


# ---------------------------------------------------------------------------
_BUILD_CACHE = {}


def _build_bass(ni, img_h, img_w):
    """Dense kernel: per instance, mx = maskT.T @ X  then out rows in 128-row
    tiles: out = YtT.T @ mx."""
    import concourse.bass as bass
    import concourse.mybir as mybir
    from concourse.tile import TileContext

    f32 = mybir.dt.float32
    nc = bass.Bass()
    maskT_d = nc.dram_tensor("maskT", [ni, WM, HM], f32, kind="ExternalInput")
    x_d = nc.dram_tensor("xmat", [ni, WM, img_w], f32, kind="ExternalInput")
    yt_d = nc.dram_tensor("ytmat", [ni, HM, img_h], f32, kind="ExternalInput")
    out_d = nc.dram_tensor("out", [ni, img_h, img_w], f32, kind="ExternalOutput")

    chunks = []
    c = 0
    while c < img_w:
        cw = min(512, img_w - c)
        chunks.append((c, cw))
        c += cw
    rtiles = []
    r = 0
    while r < img_h:
        rh = min(128, img_h - r)
        rtiles.append((r, rh))
        r += rh

    with TileContext(nc) as tc:
        with (
            tc.tile_pool(name="w", bufs=3) as wp,
            tc.tile_pool(name="mx", bufs=2) as mxp,
            tc.tile_pool(name="psA", bufs=2, space="PSUM") as psa,
            tc.tile_pool(name="psB", bufs=2, space="PSUM") as psb,
            tc.tile_pool(name="ob", bufs=3) as obp,
        ):
            for n in range(ni):
                mT = wp.tile([WM, HM], f32, tag="mT")
                xt = wp.tile([WM, img_w], f32, tag="xt")
                yt = wp.tile([HM, img_h], f32, tag="yt")
                nc.sync.dma_start(out=mT[:], in_=maskT_d[n])
                nc.sync.dma_start(out=xt[:], in_=x_d[n])
                nc.sync.dma_start(out=yt[:], in_=yt_d[n])

                mx = mxp.tile([HM, img_w], f32, tag="mx")
                for j, (c0, cw) in enumerate(chunks):
                    pa = psa.tile([HM, 512], f32, tag="pa")
                    nc.tensor.matmul(
                        out=pa[:, :cw],
                        lhsT=mT[:],
                        rhs=xt[:, c0 : c0 + cw],
                        start=True,
                        stop=True,
                    )
                    if j % 2 == 0:
                        nc.vector.tensor_copy(out=mx[:, c0 : c0 + cw], in_=pa[:, :cw])
                    else:
                        nc.scalar.copy(out=mx[:, c0 : c0 + cw], in_=pa[:, :cw])

                for r0, rh in rtiles:
                    pb = psb.tile([128, 3 * 512], f32, tag="pb")
                    for j, (c0, cw) in enumerate(chunks):
                        nc.tensor.matmul(
                            out=pb[:rh, j * 512 : j * 512 + cw],
                            lhsT=yt[:, r0 : r0 + rh],
                            rhs=mx[:, c0 : c0 + cw],
                            start=True,
                            stop=True,
                        )
                    ob = obp.tile([128, img_w], f32, tag="ob")
                    # evacuate PSUM -> SBUF split across DVE and ACT
                    half = (img_w // 2 + 63) & ~63
                    # psum has chunks at 512-stride; copy per chunk, alternating
                    for j, (c0, cw) in enumerate(chunks):
                        eng = nc.vector.tensor_copy if j % 2 == 0 else nc.scalar.copy
                        eng(
                            out=ob[:rh, c0 : c0 + cw],
                            in_=pb[:rh, j * 512 : j * 512 + cw],
                        )
                    nc.sync.dma_start(
                        out=out_d[n, r0 : r0 + rh, :], in_=ob[:rh, :]
                    )
    _split_multi_waits(nc)
    return nc


def _prep_inputs(masks, boxes, img_h, img_w, in_h, in_w):
    n = masks.shape[0]
    ni = n // N_CORES
    x0, y0, x1, y1 = _scaled_boxes(boxes, img_h, img_w, in_h, in_w)
    xmat = _interp_mats(x0, x1, img_w, WM)  # [N, 28, img_w]
    ytmat = _interp_mats(y0, y1, img_h, HM)  # [N, 28, img_h] (= Y^T per inst)
    maskt = np.ascontiguousarray(
        np.transpose(masks[:, 0].astype(np.float32), (0, 2, 1))
    )  # [N, 28, 28]
    in_maps = []
    for c in range(N_CORES):
        s = slice(c * ni, (c + 1) * ni)
        in_maps.append(
            {
                "maskT": maskt[s],
                "xmat": xmat[s],
                "ytmat": ytmat[s],
            }
        )
    return in_maps


def _run(masks, boxes, img_h, img_w, in_h, in_w, trace=False):
    from concourse.bass_utils import run_bass_kernel_spmd

    n = masks.shape[0]
    assert n % N_CORES == 0
    ni = n // N_CORES
    key = (ni, img_h, img_w)
    if key not in _BUILD_CACHE:
        _BUILD_CACHE[key] = _build_bass(ni, img_h, img_w)
    nc = _BUILD_CACHE[key]
    in_maps = _prep_inputs(masks, boxes, img_h, img_w, in_h, in_w)
    res = run_bass_kernel_spmd(nc, in_maps, core_ids=list(range(N_CORES)), trace=trace)
    out = np.concatenate([res.results[c]["out"] for c in range(N_CORES)], axis=0)
    return out, res


def kernel(masks, boxes, img_h, img_w, in_h, in_w):
    img_h = int(img_h)
    img_w = int(img_w)
    in_h = int(in_h)
    in_w = int(in_w)
    masks = np.asarray(masks, dtype=np.float32)
    boxes = np.asarray(boxes, dtype=np.float32)
    out, _ = _run(masks, boxes, img_h, img_w, in_h, in_w, trace=False)
    return out
